# revision 14
# baseline (speedup 1.0000x reference)
"""Adaptive average pooling (8,384,384,64) NHWC -> (8,7,7,64) on 8 TRN2 NeuronCores.

Pure data parallel: one batch sample per core, no collectives. Per core:
  - Stream the sample as 21 slabs (7 adaptive W-windows x 3 H-chunks), each
    (128 h x 56 w x 64 c), via SWDGE DMAs that cast f32 -> bf16 in flight.
    All 21 bf16 slabs (150.5 KB/partition) stay resident in SBUF - no ring
    reuse, so no write-after-read hazards on the DMAs.
  - TensorEngine reduces over H (the partition dim) with bf16 matmuls: for
    each W-window j and H-chunk k the stationary P_{j,k} (128 x 7) is an
    exact 0/1 bf16 membership mask of the H-windows, and the moving operand
    is a strided slab view (c-block 8, w) so PSUM fills in (chunk, c', w)
    layout: psum[i, cb*8+c', w] = sum_h P[h,i] * x[h, w, c]. bf16 streams
    1 row/cycle; fp32 would be 4x slower and gate the kernel.
  - Per window, one DVE tensor_reduce over the contiguous w axis of PSUM
    (fp32 accumulations) -> y_raw[:, j*64:(j+1)*64].
  - One DVE multiply applies the exact fp32 1/(sh_i*sw_j) scaling table,
    then one DMA out (7 x 448).

Raw Bass blocks with explicit semaphores (TileContext's generated sync
exceeds this toolchain's per-instruction sync-wait limits). Memory-bound:
~37.75 MB HBM reads per core (~91 us); PE ~70-85 us and DVE ~19 us hide
under the DMA stream.
"""

import numpy as np
import ml_dtypes

import concourse.bass as bass
import concourse.mybir as mybir
from concourse.bass_utils import run_bass_kernel_spmd

B, H, W, C = 8, 384, 384, 64
OUT = 7
N_CORES = 8
KH = H // 128  # 3 H-chunks of 128 rows
WMAX = 56  # largest adaptive window along W
NSLAB = OUT * KH  # 21 slabs per core, window-major: n = j*KH + k
SLAB = WMAX * C  # 3584 elements per partition line
CB = 8  # c-block width per PSUM bank chunk; 64/8 = 8 chunks per window

_F32 = mybir.dt.float32
_BF16 = mybir.dt.bfloat16


def _windows(d, out):
    starts = np.floor(np.arange(out) * d / out).astype(np.int64)
    ends = np.ceil((np.arange(out) + 1) * d / out).astype(np.int64)
    return starts, ends - starts


def _build():
    nc = bass.Bass()
    x = nc.declare_dram_parameter("x", [H, W * C], _F32, isOutput=False)
    pmat = nc.declare_dram_parameter(
        "pmat", [128, NSLAB * OUT], _BF16, isOutput=False
    )
    invhw = nc.declare_dram_parameter("invhw", [OUT, OUT * C], _F32, isOutput=False)
    out = nc.declare_dram_parameter("out", [OUT, OUT * C], _F32, isOutput=True)

    ws, wsz = _windows(W, OUT)
    # Read a full WMAX-wide slab for every window (uniform shape); matmul APs
    # select the exact window inside it. s_read/off handle the last window,
    # whose 56-wide read would run off the right edge.
    reads = []
    for j in range(OUT):
        s, sz = int(ws[j]), int(wsz[j])
        s_read = min(s, W - WMAX)
        reads.append((s_read, s - s_read, sz))

    with (
        nc.sbuf_tensor([128, NSLAB * SLAB], _BF16) as xbuf,
        nc.sbuf_tensor([128, NSLAB * OUT], _BF16) as p_sb,
        nc.sbuf_tensor([OUT, OUT * C], _F32) as invhw_sb,
        nc.sbuf_tensor([OUT, OUT * C], _F32) as y_raw,
        nc.sbuf_tensor([OUT, OUT * C], _F32) as y_sb,
        nc.psum_tensor([128, CB * 512], _F32) as psum,
        nc.semaphore("win_sem") as win_sem,
        nc.semaphore("dve_sem") as dve_sem,
        nc.semaphore("const_sem") as const_sem,
        nc.semaphore("mul_sem") as mul_sem,
        nc.semaphore("out_sem") as out_sem,
    ):
        slab_sems = [nc.alloc_semaphore(f"slab{n}") for n in range(NSLAB)]

        with nc.Block() as block:

            @block.gpsimd
            def _(gpsimd):
                # consts first, then the 21 casting slab DMAs (SWDGE only —
                # HWDGE engines cannot cast f32->bf16 in flight)
                gpsimd.dma_start(out=p_sb[:], in_=pmat[:]).then_inc(const_sem, 16)
                gpsimd.dma_start(out=invhw_sb[:], in_=invhw[:]).then_inc(
                    const_sem, 16
                )
                for n in range(NSLAB):
                    j, k = divmod(n, KH)
                    s_read = reads[j][0]
                    gpsimd.dma_start(
                        out=xbuf[:, n * SLAB : (n + 1) * SLAB],
                        in_=x[
                            k * 128 : (k + 1) * 128,
                            s_read * C : (s_read + WMAX) * C,
                        ],
                    ).then_inc(slab_sems[n], 16)

            @block.tensor
            def _(tensor):
                tensor.wait_ge(const_sem, 32)
                for j in range(OUT):
                    _, off, sz = reads[j]
                    if j > 0:
                        # previous window's PSUM must be drained by the DVE
                        tensor.wait_ge(dve_sem, j)
                    for k in range(KH):
                        n = j * KH + k
                        tensor.wait_ge(slab_sems[n], 16)
                        base = n * SLAB + off * C
                        win = xbuf[:, base : base + sz * C].rearrange(
                            "p (w c) -> p c w", c=C
                        )
                        lhsT = p_sb[:, n * OUT : (n + 1) * OUT]
                        for cb in range(CB):
                            mm = tensor.matmul(
                                psum[:OUT, cb * 512 : cb * 512 + CB * sz],
                                lhsT,
                                win[:, cb * CB : (cb + 1) * CB, :],
                                start=(k == 0),
                                stop=(k == KH - 1),
                            )
                    mm.then_inc(win_sem, 1)

            @block.vector
            def _(vector):
                for j in range(OUT):
                    _, _, sz = reads[j]
                    vector.wait_ge(win_sem, j + 1)
                    src = (
                        psum[:OUT, :]
                        .rearrange("p (n x) -> p n x", n=CB)[:, :, : CB * sz]
                        .rearrange("p n (c w) -> p n c w", c=CB)
                    )
                    vector.tensor_reduce(
                        out=y_raw[:, j * C : (j + 1) * C].rearrange(
                            "p (n c) -> p n c", n=CB
                        ),
                        in_=src,
                        axis=mybir.AxisListType.X,
                        op=mybir.AluOpType.add,
                    ).then_inc(dve_sem, 1)
                vector.wait_ge(const_sem, 32)
                vector.wait_ge(dve_sem, OUT)
                vector.tensor_mul(y_sb[:], y_raw[:], invhw_sb[:]).then_inc(
                    mul_sem, 1
                )

            @block.sync
            def _(sync):
                sync.wait_ge(mul_sem, 1)
                sync.dma_start(out=out[:], in_=y_sb[:]).then_inc(out_sem, 16)
                sync.wait_ge(out_sem, 16)

    return nc


def _consts():
    hs, hsz = _windows(H, OUT)
    _, wsz = _windows(W, OUT)
    p = np.zeros((128, NSLAB * OUT), np.float32)
    for j in range(OUT):
        for k in range(KH):
            n = j * KH + k
            for i in range(OUT):
                h0, h1 = int(hs[i]), int(hs[i] + hsz[i])
                for h in range(max(h0, k * 128), min(h1, (k + 1) * 128)):
                    p[h - k * 128, n * OUT + i] = 1.0
    inv = np.zeros((OUT, OUT * C), np.float32)
    for i in range(OUT):
        for j in range(OUT):
            inv[i, j * C : (j + 1) * C] = 1.0 / (float(hsz[i]) * float(wsz[j]))
    return p.astype(ml_dtypes.bfloat16), inv


_NC_CACHE = None


def _run(x, **kwargs):
    global _NC_CACHE
    if _NC_CACHE is None:
        _NC_CACHE = _build()
    nc = _NC_CACHE
    p, inv = _consts()
    x = np.ascontiguousarray(np.asarray(x, dtype=np.float32))
    in_maps = [
        {"x": x[b].reshape(H, W * C), "pmat": p, "invhw": inv}
        for b in range(N_CORES)
    ]
    res = run_bass_kernel_spmd(nc, in_maps, core_ids=list(range(N_CORES)), **kwargs)
    y = np.stack(
        [res.results[b]["out"].reshape(OUT, OUT, C) for b in range(N_CORES)]
    )
    return y, res


def kernel(x: np.ndarray) -> np.ndarray:
    y, _ = _run(x)
    return y


# revision 16
# speedup vs baseline: 1.2821x; 1.2821x over previous
"""Adaptive average pooling (8,384,384,64) NHWC -> (8,7,7,64) on 8 TRN2 NeuronCores.

Pure data parallel: one batch sample per core, no collectives. Per core:
  - Stream the sample as 21 slabs (7 adaptive W-windows x 3 H-chunks), each
    (128 h x 56 w x 64 c), via SWDGE DMAs that cast f32 -> bf16 in flight
    (the cast is line-rate in the DMA datapath; HBM reads stay f32-sized).
    Slabs live in a 14-deep bf16 SBUF ring.
  - TensorEngine reduces over H (the partition dim) with bf16 matmuls: for
    each W-window j and H-chunk k the stationary P_{j,k} (128 x 7) is an
    exact 0/1 bf16 membership mask of the H-windows; the moving operand is
    a CONTIGUOUS 512-column slab slice (8 w x 64 c) - strided rhs runs at
    ~2.4 cyc/row, contiguous at 1 - so each window accumulates into PSUM
    banks 0-6 as psum[i, cb, w', c] = sum_h P[h,i] * x[h, w, c].
  - ScalarEngine (ACT) drains each PSUM bank to bf16 SBUF right after its
    stop-matmul, so the PE's next window never waits on a full-window drain.
  - DVE reduces each drained window over (cb, w') with a strided XY
    tensor_reduce (25x less data than stage 1, so the stride penalty is
    hidden), subtracts the one out-of-window column for the two 55-wide
    windows, applies the exact fp32 1/(sh_i*sw_j) table, DMA out (7 x 448).

Raw Bass blocks with explicit semaphores (TileContext's generated sync
exceeds this toolchain's per-instruction sync-wait limits). Memory-bound:
~37.75 MB HBM reads per core (~91 us); PE ~70 us, ACT ~27 us, DVE ~45 us
all hide under the DMA stream.
"""

import numpy as np
import ml_dtypes

import concourse.bass as bass
import concourse.mybir as mybir
from concourse.bass_utils import run_bass_kernel_spmd

B, H, W, C = 8, 384, 384, 64
OUT = 7
N_CORES = 8
KH = H // 128  # 3 H-chunks of 128 rows
WMAX = 56  # uniform slab width along W
NSLAB = OUT * KH  # 21 slabs per core, window-major: n = j*KH + k
SLAB = WMAX * C  # 3584 elements per partition line
RING = 14  # SBUF ring depth in slabs
NCH = 7  # 512-col chunks per window (7 x (8 w x 64 c) = 3584)
WIN = NCH * 512  # psum/t columns per window

_F32 = mybir.dt.float32
_BF16 = mybir.dt.bfloat16


def _windows(d, out):
    starts = np.floor(np.arange(out) * d / out).astype(np.int64)
    ends = np.ceil((np.arange(out) + 1) * d / out).astype(np.int64)
    return starts, ends - starts


def _reads():
    ws, wsz = _windows(W, OUT)
    reads = []
    for j in range(OUT):
        s, sz = int(ws[j]), int(wsz[j])
        s_read = min(s, W - WMAX)
        # garbage column (slab-local w') for sz=55 windows, else None
        garb = None
        if sz < WMAX:
            garb = 0 if s - s_read > 0 else WMAX - 1
        reads.append((s_read, garb))
    return reads


def _build():
    nc = bass.Bass()
    x = nc.declare_dram_parameter("x", [H, W * C], _F32, isOutput=False)
    pmat = nc.declare_dram_parameter(
        "pmat", [128, NSLAB * OUT], _BF16, isOutput=False
    )
    invhw = nc.declare_dram_parameter("invhw", [OUT, OUT * C], _F32, isOutput=False)
    out = nc.declare_dram_parameter("out", [OUT, OUT * C], _F32, isOutput=True)

    reads = _reads()

    with (
        nc.sbuf_tensor([128, RING * SLAB], _BF16) as xbuf,
        nc.sbuf_tensor([128, NSLAB * OUT], _BF16) as p_sb,
        nc.sbuf_tensor([OUT, OUT * WIN], _BF16) as t_sb,
        nc.sbuf_tensor([OUT, OUT * C], _F32) as invhw_sb,
        nc.sbuf_tensor([OUT, OUT * C], _F32) as y_raw,
        nc.sbuf_tensor([OUT, OUT * C], _F32) as y_sb,
        nc.psum_tensor([128, NCH * 512], _F32) as psum,
        nc.semaphore("const_sem") as const_sem,
        nc.semaphore("pe_slab_sem") as pe_slab_sem,
        nc.semaphore("chunk_sem") as chunk_sem,
        nc.semaphore("drain_sem") as drain_sem,
        nc.semaphore("dve_sem") as dve_sem,
        nc.semaphore("mul_sem") as mul_sem,
        nc.semaphore("out_sem") as out_sem,
    ):
        slab_sems = [nc.alloc_semaphore(f"slab{n}") for n in range(NSLAB)]

        with nc.Block() as block:

            @block.gpsimd
            def _(gpsimd):
                gpsimd.dma_start(out=p_sb[:], in_=pmat[:]).then_inc(const_sem, 16)
                gpsimd.dma_start(out=invhw_sb[:], in_=invhw[:]).then_inc(
                    const_sem, 16
                )
                for n in range(NSLAB):
                    j, k = divmod(n, KH)
                    s_read = reads[j][0]
                    r = n % RING
                    if n >= RING:
                        # WAR: slot's previous slab fully consumed by the PE.
                        # Stop-passes (k==KH-1) signal via chunk_sem instead
                        # (one sem update per instruction).
                        jp, kp = divmod(n - RING, KH)
                        if kp == KH - 1:
                            gpsimd.wait_ge(chunk_sem, (jp + 1) * NCH)
                        else:
                            gpsimd.wait_ge(pe_slab_sem, jp * (KH - 1) + kp + 1)
                    gpsimd.dma_start(
                        out=xbuf[:, r * SLAB : (r + 1) * SLAB],
                        in_=x[
                            k * 128 : (k + 1) * 128,
                            s_read * C : (s_read + WMAX) * C,
                        ],
                    ).then_inc(slab_sems[n], 16)

            @block.tensor
            def _(tensor):
                tensor.wait_ge(const_sem, 32)
                for j in range(OUT):
                    for k in range(KH):
                        n = j * KH + k
                        r = n % RING
                        tensor.wait_ge(slab_sems[n], 16)
                        lhsT = p_sb[:, n * OUT : (n + 1) * OUT]
                        for cb in range(NCH):
                            if j > 0 and k == 0:
                                # WAR: previous window's bank cb drained
                                tensor.wait_ge(drain_sem, (j - 1) * NCH + cb + 1)
                            mm = tensor.matmul(
                                psum[:OUT, cb * 512 : (cb + 1) * 512],
                                lhsT,
                                xbuf[
                                    :,
                                    r * SLAB + cb * 512 : r * SLAB + (cb + 1) * 512,
                                ],
                                start=(k == 0),
                                stop=(k == KH - 1),
                            )
                            if k == KH - 1:
                                mm.then_inc(chunk_sem, 1)
                        if k < KH - 1:
                            mm.then_inc(pe_slab_sem, 1)

            @block.scalar
            def _(scalar):
                # drain each finished PSUM bank to bf16 SBUF
                for j in range(OUT):
                    for cb in range(NCH):
                        scalar.wait_ge(chunk_sem, j * NCH + cb + 1)
                        scalar.copy(
                            out=t_sb[
                                :, j * WIN + cb * 512 : j * WIN + (cb + 1) * 512
                            ],
                            in_=psum[:OUT, cb * 512 : (cb + 1) * 512],
                        ).then_inc(drain_sem, 1)

            @block.vector
            def _(vector):
                ticks = 0
                for j in range(OUT):
                    vector.wait_ge(drain_sem, NCH * (j + 1))
                    src = t_sb[:, j * WIN : (j + 1) * WIN].rearrange(
                        "p (n w c) -> p c n w", n=NCH, c=C
                    )
                    vector.tensor_reduce(
                        out=y_raw[:, j * C : (j + 1) * C],
                        in_=src,
                        axis=mybir.AxisListType.XY,
                        op=mybir.AluOpType.add,
                    ).then_inc(dve_sem, 1)
                    ticks += 1
                    garb = reads[j][1]
                    if garb is not None:
                        # remove the one slab column outside the true window
                        vector.wait_ge(dve_sem, ticks)
                        g0 = j * WIN + garb * C
                        vector.tensor_sub(
                            y_raw[:, j * C : (j + 1) * C],
                            y_raw[:, j * C : (j + 1) * C],
                            t_sb[:, g0 : g0 + C],
                        ).then_inc(dve_sem, 1)
                        ticks += 1
                vector.wait_ge(const_sem, 32)
                vector.wait_ge(dve_sem, ticks)
                vector.tensor_mul(y_sb[:], y_raw[:], invhw_sb[:]).then_inc(
                    mul_sem, 1
                )

            @block.sync
            def _(sync):
                sync.wait_ge(mul_sem, 1)
                sync.dma_start(out=out[:], in_=y_sb[:]).then_inc(out_sem, 16)
                sync.wait_ge(out_sem, 16)

    return nc


def _consts():
    hs, hsz = _windows(H, OUT)
    _, wsz = _windows(W, OUT)
    p = np.zeros((128, NSLAB * OUT), np.float32)
    for j in range(OUT):
        for k in range(KH):
            n = j * KH + k
            for i in range(OUT):
                h0, h1 = int(hs[i]), int(hs[i] + hsz[i])
                for h in range(max(h0, k * 128), min(h1, (k + 1) * 128)):
                    p[h - k * 128, n * OUT + i] = 1.0
    inv = np.zeros((OUT, OUT * C), np.float32)
    for i in range(OUT):
        for j in range(OUT):
            inv[i, j * C : (j + 1) * C] = 1.0 / (float(hsz[i]) * float(wsz[j]))
    return p.astype(ml_dtypes.bfloat16), inv


_NC_CACHE = None


def _run(x, **kwargs):
    global _NC_CACHE
    if _NC_CACHE is None:
        _NC_CACHE = _build()
    nc = _NC_CACHE
    p, inv = _consts()
    x = np.ascontiguousarray(np.asarray(x, dtype=np.float32))
    in_maps = [
        {"x": x[b].reshape(H, W * C), "pmat": p, "invhw": inv}
        for b in range(N_CORES)
    ]
    res = run_bass_kernel_spmd(nc, in_maps, core_ids=list(range(N_CORES)), **kwargs)
    y = np.stack(
        [res.results[b]["out"].reshape(OUT, OUT, C) for b in range(N_CORES)]
    )
    return y, res


def kernel(x: np.ndarray) -> np.ndarray:
    y, _ = _run(x)
    return y


# revision 17
# speedup vs baseline: 1.2945x; 1.0096x over previous
"""Adaptive average pooling (8,384,384,64) NHWC -> (8,7,7,64) on 8 TRN2 NeuronCores.

Pure data parallel: one batch sample per core, no collectives. Per core:
  - Stream the sample as 12 span-slabs (4 overlapping W-spans x 3 H-chunks,
    spans [0,110) [109,220) [219,330) [328,384) so each span contains two
    whole adaptive W-windows) via SWDGE DMAs that cast f32 -> bf16 in
    flight. Big slabs amortize the ~3.5 us/DMA SWDGE descriptor-generation
    cost that starved a 21-DMA version. Slabs live in a 9-slot SBUF ring.
  - TensorEngine reduces over H (the partition dim) with bf16 matmuls: for
    each W-window j and H-chunk k the stationary P_{j,k} (128 x 7) is an
    exact 0/1 bf16 membership mask of the H-windows; the moving operand is
    a CONTIGUOUS 512-column slice (8 w x 64 c) of the window's 56-wide view
    (strided rhs runs at ~2.4 cyc/row, contiguous at 1), accumulating into
    PSUM banks 0-6 as psum[i, cb, w', c] = sum_h P[h,i] * x[h, w, c].
  - ScalarEngine (ACT) drains each PSUM bank to a 2-window bf16 SBUF ring
    right after its stop-matmul, so the PE's next window never waits on a
    full-window drain.
  - DVE reduces each drained window over (cb, w') with a strided XY
    tensor_reduce (25x less data than stage 1, so the stride penalty is
    hidden), subtracts the one out-of-window column for the two 55-wide
    windows, applies the exact fp32 1/(sh_i*sw_j) table, DMA out (7 x 448).

Raw Bass blocks with explicit semaphores (TileContext's generated sync
exceeds this toolchain's per-instruction sync-wait limits). Memory-bound:
~37.9 MB HBM reads per core (~90 us at the 16x27 GB/s SDMA fabric rate);
PE ~56 us, ACT ~30 us, DVE ~45 us, GpSimd issue ~45 us all hide under it.
"""

import numpy as np
import ml_dtypes

import concourse.bass as bass
import concourse.mybir as mybir
from concourse.bass_utils import run_bass_kernel_spmd

B, H, W, C = 8, 384, 384, 64
OUT = 7
N_CORES = 8
KH = H // 128  # 3 H-chunks of 128 rows
WMAX = 56  # uniform per-window view width along W
NCH = 7  # 512-col chunks per window (7 x (8 w x 64 c) = 3584)
WIN = NCH * 512  # t columns per window
SPANS = [(0, 110), (109, 111), (219, 111), (328, 56)]  # (w_start, width)
NSPAN = len(SPANS)  # 4 W-spans, each holding 2 windows (last: 1)
NSLAB = NSPAN * KH  # 12 DMA slabs, span-major: s = g*KH + k
SLOT = 111 * C  # ring slot size (max span width)
RING = 9  # slab ring depth
# per window: (span g, local w-offset of its 56-wide view, garbage column
# inside that view or None) — windows 0 and 6 are 55 wide and carry one
# out-of-window column whose contribution is subtracted at the end.
WINDOWS = [
    (0, 0, 55),
    (0, 54, None),
    (1, 0, None),
    (1, 55, None),
    (2, 0, None),
    (2, 55, None),
    (3, 0, 0),
]

_F32 = mybir.dt.float32
_BF16 = mybir.dt.bfloat16


def _windows(d, out):
    starts = np.floor(np.arange(out) * d / out).astype(np.int64)
    ends = np.ceil((np.arange(out) + 1) * d / out).astype(np.int64)
    return starts, ends - starts


def _build():
    nc = bass.Bass()
    x = nc.declare_dram_parameter("x", [H, W * C], _F32, isOutput=False)
    pmat = nc.declare_dram_parameter(
        "pmat", [128, OUT * KH * OUT], _BF16, isOutput=False
    )
    invhw = nc.declare_dram_parameter("invhw", [OUT, OUT * C], _F32, isOutput=False)
    out = nc.declare_dram_parameter("out", [OUT, OUT * C], _F32, isOutput=True)

    with (
        nc.sbuf_tensor([128, RING * SLOT], _BF16) as xbuf,
        nc.sbuf_tensor([128, OUT * KH * OUT], _BF16) as p_sb,
        nc.sbuf_tensor([OUT, 2 * WIN], _BF16) as t_sb,
        nc.sbuf_tensor([OUT, OUT * C], _F32) as invhw_sb,
        nc.sbuf_tensor([OUT, OUT * C], _F32) as y_raw,
        nc.sbuf_tensor([OUT, OUT * C], _F32) as y_sb,
        nc.psum_tensor([128, NCH * 512], _F32) as psum,
        nc.semaphore("const_sem") as const_sem,
        nc.semaphore("pe_pass_sem") as pe_pass_sem,
        nc.semaphore("chunk_sem") as chunk_sem,
        nc.semaphore("drain_sem") as drain_sem,
        nc.semaphore("dve_sem") as dve_sem,
        nc.semaphore("mul_sem") as mul_sem,
        nc.semaphore("out_sem") as out_sem,
    ):
        slab_sems = [nc.alloc_semaphore(f"slab{s}") for s in range(NSLAB)]

        # dve_sem ticks accumulated after window j's reduce (+subtract)
        dve_ticks_after = []
        t = 0
        for j in range(OUT):
            t += 1 + (1 if WINDOWS[j][2] is not None else 0)
            dve_ticks_after.append(t)

        def pass_wait(eng, j, k):
            """Wait until the PE finished pass (j, k) (window j, H-chunk k)."""
            if k == KH - 1:
                eng.wait_ge(chunk_sem, (j + 1) * NCH)
            else:
                eng.wait_ge(pe_pass_sem, j * (KH - 1) + k + 1)

        with nc.Block() as block:

            @block.gpsimd
            def _(gpsimd):
                gpsimd.dma_start(out=p_sb[:], in_=pmat[:]).then_inc(const_sem, 16)
                gpsimd.dma_start(out=invhw_sb[:], in_=invhw[:]).then_inc(
                    const_sem, 16
                )
                for s in range(NSLAB):
                    g, k = divmod(s, KH)
                    w0, wd = SPANS[g]
                    r = s % RING
                    if s >= RING:
                        # WAR: slot's previous slab (span g', chunk k') was
                        # last read by the PE pass (last window of g', k').
                        gp, kp = divmod(s - RING, KH)
                        pass_wait(gpsimd, min(2 * gp + 1, OUT - 1), kp)
                    gpsimd.dma_start(
                        out=xbuf[:, r * SLOT : r * SLOT + wd * C],
                        in_=x[k * 128 : (k + 1) * 128, w0 * C : (w0 + wd) * C],
                    ).then_inc(slab_sems[s], 16)

            @block.tensor
            def _(tensor):
                tensor.wait_ge(const_sem, 32)
                for j in range(OUT):
                    g, off, _ = WINDOWS[j]
                    for k in range(KH):
                        s = g * KH + k
                        r = s % RING
                        if j == 2 * g:  # first window of the span
                            tensor.wait_ge(slab_sems[s], 16)
                        n = j * KH + k
                        lhsT = p_sb[:, n * OUT : (n + 1) * OUT]
                        base = r * SLOT + off * C
                        for cb in range(NCH):
                            if j > 0 and k == 0:
                                # WAR: previous window's bank cb drained
                                tensor.wait_ge(drain_sem, (j - 1) * NCH + cb + 1)
                            mm = tensor.matmul(
                                psum[:OUT, cb * 512 : (cb + 1) * 512],
                                lhsT,
                                xbuf[:, base + cb * 512 : base + (cb + 1) * 512],
                                start=(k == 0),
                                stop=(k == KH - 1),
                            )
                            if k == KH - 1:
                                mm.then_inc(chunk_sem, 1)
                        if k < KH - 1:
                            mm.then_inc(pe_pass_sem, 1)

            @block.scalar
            def _(scalar):
                # drain each finished PSUM bank to bf16 SBUF (2-window ring)
                for j in range(OUT):
                    t0 = (j % 2) * WIN
                    for cb in range(NCH):
                        if j >= 2 and cb == 0:
                            # WAR: window j-2's reduce (and garbage-column
                            # subtract) must have consumed this t slot.
                            scalar.wait_ge(dve_sem, dve_ticks_after[j - 2])
                        scalar.wait_ge(chunk_sem, j * NCH + cb + 1)
                        scalar.copy(
                            out=t_sb[:, t0 + cb * 512 : t0 + (cb + 1) * 512],
                            in_=psum[:OUT, cb * 512 : (cb + 1) * 512],
                        ).then_inc(drain_sem, 1)

            @block.vector
            def _(vector):
                ticks = 0
                for j in range(OUT):
                    t0 = (j % 2) * WIN
                    vector.wait_ge(drain_sem, NCH * (j + 1))
                    src = t_sb[:, t0 : t0 + WIN].rearrange(
                        "p (n w c) -> p c n w", n=NCH, c=C
                    )
                    vector.tensor_reduce(
                        out=y_raw[:, j * C : (j + 1) * C],
                        in_=src,
                        axis=mybir.AxisListType.XY,
                        op=mybir.AluOpType.add,
                    ).then_inc(dve_sem, 1)
                    ticks += 1
                    garb = WINDOWS[j][2]
                    if garb is not None:
                        # remove the one slab column outside the true window
                        vector.wait_ge(dve_sem, ticks)
                        g0 = t0 + garb * C
                        vector.tensor_sub(
                            y_raw[:, j * C : (j + 1) * C],
                            y_raw[:, j * C : (j + 1) * C],
                            t_sb[:, g0 : g0 + C],
                        ).then_inc(dve_sem, 1)
                        ticks += 1
                vector.wait_ge(const_sem, 32)
                vector.wait_ge(dve_sem, ticks)
                vector.tensor_mul(y_sb[:], y_raw[:], invhw_sb[:]).then_inc(
                    mul_sem, 1
                )

            @block.sync
            def _(sync):
                sync.wait_ge(mul_sem, 1)
                sync.dma_start(out=out[:], in_=y_sb[:]).then_inc(out_sem, 16)
                sync.wait_ge(out_sem, 16)

    return nc


def _consts():
    hs, hsz = _windows(H, OUT)
    _, wsz = _windows(W, OUT)
    p = np.zeros((128, OUT * KH * OUT), np.float32)
    for j in range(OUT):
        for k in range(KH):
            n = j * KH + k
            for i in range(OUT):
                h0, h1 = int(hs[i]), int(hs[i] + hsz[i])
                for h in range(max(h0, k * 128), min(h1, (k + 1) * 128)):
                    p[h - k * 128, n * OUT + i] = 1.0
    inv = np.zeros((OUT, OUT * C), np.float32)
    for i in range(OUT):
        for j in range(OUT):
            inv[i, j * C : (j + 1) * C] = 1.0 / (float(hsz[i]) * float(wsz[j]))
    return p.astype(ml_dtypes.bfloat16), inv


_NC_CACHE = None


def _run(x, **kwargs):
    global _NC_CACHE
    if _NC_CACHE is None:
        _NC_CACHE = _build()
    nc = _NC_CACHE
    p, inv = _consts()
    x = np.ascontiguousarray(np.asarray(x, dtype=np.float32))
    in_maps = [
        {"x": x[b].reshape(H, W * C), "pmat": p, "invhw": inv}
        for b in range(N_CORES)
    ]
    res = run_bass_kernel_spmd(nc, in_maps, core_ids=list(range(N_CORES)), **kwargs)
    y = np.stack(
        [res.results[b]["out"].reshape(OUT, OUT, C) for b in range(N_CORES)]
    )
    return y, res


def kernel(x: np.ndarray) -> np.ndarray:
    y, _ = _run(x)
    return y
